# revision 13
# baseline (speedup 1.0000x reference)
"""Varlen causal GQA attention on 8 TRN2 NeuronCores.

Problem: 32 q heads, 8 kv heads, head_dim 128, ragged batch (cu_seqlens),
f32. Sharded by KV-head group: core c owns kv head c and q heads
4c..4c+3 -- fully data-independent across cores, no collectives.

Per core, blockwise causal attention in 128x128 blocks with all 4 q
heads fused through 3D access patterns (q stored head-interleaved
[d, h, t]). Work per 128-col k-block j against q-block g:
    S[k, h, q]  = (K_j)^T.T @ Q^T      one matmul per (g, j) block
    P = exp(S * scale)                 ONE ScalarE exp per chunk of
                                       CH=3 j-blocks (ACT is the
                                       critical engine: ~51us floor)
    causal mask: 0/1 multiply on GpSimd (diagonal blocks only)
    O^T[h] += V_j @ P                  one matmul, PSUM-accumulated
    softmax denominators: mixed strategy balancing PE vs DVE:
      small groups (<= ACC_MIN blocks): ones^T @ P per block (PE)
      large groups: P accumulated across j on DVE/GpSimd in bf16,
        then ONE ones^T @ acc matmul per group
Engine budget per core (95 blocks): ACT 41 exp instrs ~51us,
PE ~215 matmuls ~50us, DVE adds+PSUM-evac copies ~49us, GpSimd
masks+first-adds ~47us -- all co-binding near the ACT floor.

The instruction stream is software-pipelined at ISSUE level: AV/sums
trail their S/exp by LAG chunks so the PE FIFO never parks on an exp
wait. Scalar ring issues NO DMAs (exp only); all input DMAs ride the
Sync ring in 3 tiers (first 2 blocks -> rest of seq 0 -> per-seq bulk)
so the first matmul starts ~2us in; outputs stream out per ~4-group
slab (per-group for the last seq) to keep the drain tail short. Host
does all transposes (Q^T/K^T in, O^T -> O out), bf16 conversion,
padding of ragged sequences to 128 multiples, and the final softmax
division. oT is returned in bf16 (halves output DMA bytes).
"""

import math
import os
import sys

sys.path.insert(0, "/opt/trn_rl_repo")

import ml_dtypes
import numpy as np

NUM_HEADS = 32
NUM_KV_HEADS = 8
HEAD_DIM = 128
HEADS_PER_CORE = NUM_HEADS // NUM_KV_HEADS  # 4
N_CORES = 8
BLK = 128
SCALE = 1.0 / math.sqrt(HEAD_DIM)

CH = 3        # j-blocks per chunk (PSUM: 3 banks x 2 bufs + oT 1 + sums 1)
ACC_MIN = 4   # groups with >= this many blocks use DVE-accumulated sums
LAG = 3       # chunks the back phase (AV/sums) trails the front (S/exp)
WARM = 6      # dummy matmuls to hold the PE activity monitor at 2.4 GHz
T0_BLOCKS = 2  # tier-0: first blocks of seq 0, smallest possible first DMA

_GRAPH_CACHE = {}


def _build_graph(seq_blocks):
    """Build the SPMD Bacc graph for padded per-seq block counts."""
    from concourse import bacc
    import concourse.mybir as mybir
    from concourse.tile import TileContext

    f32 = mybir.dt.float32
    bf16 = mybir.dt.bfloat16
    T = sum(seq_blocks) * BLK
    n_blocks_total = T // BLK
    H = HEADS_PER_CORE

    nc = bacc.Bacc("TRN2", target_bir_lowering=False, debug=False,
                   num_devices=N_CORES)

    qT_ext = nc.declare_dram_parameter("qT", [BLK, H, T], bf16, isOutput=False)
    kT_ext = nc.declare_dram_parameter("kT", [BLK, T], bf16, isOutput=False)
    v_ext = nc.declare_dram_parameter("v", [T, HEAD_DIM], bf16, isOutput=False)
    mask_ext = nc.declare_dram_parameter("mask", [BLK, H, BLK], bf16,
                                         isOutput=False)
    oT_ext = nc.declare_dram_parameter("oT", [BLK, H, T], bf16, isOutput=True)
    sums_ext = nc.declare_dram_parameter("sums", [1, H, T], f32, isOutput=True)

    nb0 = seq_blocks[0]
    t0 = min(T0_BLOCKS, nb0)  # tier-0 block count
    c_t0 = t0 * BLK
    c_s0 = nb0 * BLK

    with TileContext(nc) as tc:
        with (
            tc.tile_pool(name="persist", bufs=1) as persist,
            tc.tile_pool(name="p", bufs=7) as p_pool,
            tc.tile_pool(name="accp", bufs=2) as acc_pool,
            tc.tile_pool(name="ps_s", bufs=2, space="PSUM") as ps_s,
            tc.tile_pool(name="ps_o", bufs=2, space="PSUM") as ps_o,
        ):
            v_re = v_ext[:].rearrange("(j p) d -> p j d", p=BLK)

            # tier-0: the opening blocks of seq 0 in their own small tiles
            # (dependencies are tile-granular; these land in ~1.5us so the
            # first real matmul isn't gated on bulk transfers)
            kT_q0 = persist.tile([BLK, c_t0], bf16)
            qT_q0 = persist.tile([BLK, H, c_t0], bf16)
            v_q0 = persist.tile([BLK, t0, HEAD_DIM], bf16)
            mask_sb = persist.tile([BLK, H, BLK], bf16)
            nc.sync.dma_start(kT_q0[:], kT_ext[:, :c_t0])
            nc.sync.dma_start(qT_q0[:], qT_ext[:, :, :c_t0])
            nc.sync.dma_start(v_q0[:], v_re[:, :t0, :])
            nc.sync.dma_start(mask_sb[:], mask_ext[:])

            # tier-1: the rest of seq 0, issued from the Scalar ring (its
            # HWDGE queue is a second independent DMA ring; the exp stream
            # hasn't started yet so these issues stall nothing)
            kT_q1 = persist.tile([BLK, c_s0 - c_t0], bf16)
            qT_q1 = persist.tile([BLK, H, c_s0 - c_t0], bf16)
            v_q1 = persist.tile([BLK, nb0 - t0, HEAD_DIM], bf16)
            if nb0 > t0:
                nc.scalar.dma_start(kT_q1[:], kT_ext[:, c_t0:c_s0])
                nc.scalar.dma_start(v_q1[:], v_re[:, t0:nb0, :])
                nc.scalar.dma_start(qT_q1[:], qT_ext[:, :, c_t0:c_s0])

            # tier-2: remaining sequences, one set of DMAs per sequence,
            # spread across three DMA rings (sync HWDGE / scalar HWDGE /
            # gpsimd SWDGE) so no single ring's ~155 GB/s bounds the input
            kT_sb = persist.tile([BLK, max(T - c_s0, BLK)], bf16)
            qT_sb = persist.tile([BLK, H, max(T - c_s0, BLK)], bf16)
            v_sb = persist.tile(
                [BLK, max(n_blocks_total - nb0, 1), HEAD_DIM], bf16)
            off = c_s0
            for si, nblk in enumerate(seq_blocks[1:]):
                o0 = off - c_s0
                j0 = o0 // BLK
                ls = nblk * BLK
                kv_eng = nc.sync if si == 0 else nc.gpsimd
                q_eng = nc.sync if si == 0 else nc.scalar
                kv_eng.dma_start(kT_sb[:, o0:o0 + ls],
                                 kT_ext[:, off:off + ls])
                kv_eng.dma_start(v_sb[:, j0:j0 + nblk, :],
                                 v_re[:, off // BLK:off // BLK + nblk, :])
                q_eng.dma_start(qT_sb[:, :, o0:o0 + ls],
                                qT_ext[:, :, off:off + ls])
                off += ls

            ones_f = persist.tile([BLK, BLK], f32)
            nc.vector.memset(ones_f[:], 1.0)
            # full [128,128] ones stationary: sums matmul runs M=128 so the
            # PE array never reconfigures col groups between AV and sums
            ones_b = persist.tile([BLK, BLK], bf16)
            nc.vector.tensor_copy(ones_b[:], ones_f[:])

            ot_stage = persist.tile([BLK, H, T], bf16)
            sums_stage = persist.tile([1, H, T], f32)

            # HAM warm-up: dummy matmuls (no DMA deps) cover the tier-0 DMA
            # latency and keep the PE activity monitor at full clock
            warm_sb = persist.tile([BLK, 4 * BLK], bf16)
            nc.vector.memset(warm_sb[:], 1.0)
            for _ in range(WARM):
                warm_ps = ps_s.tile([BLK, CH, H, BLK], f32, tag="s3",
                                    name="warm")
                nc.tensor.matmul(
                    warm_ps[:, 0].rearrange("p h q -> p (h q)"),
                    warm_sb[:, :BLK], warm_sb[:],
                    start=True, stop=True,
                )

            def k_sl(seq_off, j):
                if seq_off == 0:
                    if j < t0:
                        return kT_q0[:, j * BLK:(j + 1) * BLK]
                    return kT_q1[:, (j - t0) * BLK:(j - t0 + 1) * BLK]
                c = seq_off - c_s0 + j * BLK
                return kT_sb[:, c:c + BLK]

            def q_sl(seq_off, g):
                if seq_off == 0:
                    if g < t0:
                        return qT_q0[:, :, g * BLK:(g + 1) * BLK]
                    return qT_q1[:, :, (g - t0) * BLK:(g - t0 + 1) * BLK]
                c = seq_off - c_s0 + g * BLK
                return qT_sb[:, :, c:c + BLK]

            def v_sl(seq_off, j):
                if seq_off == 0:
                    if j < t0:
                        return v_q0[:, j, :]
                    return v_q1[:, j - t0, :]
                return v_sb[:, (seq_off - c_s0) // BLK + j, :]

            # flat chunk stream over (seq, q-block g, k-block j-triples),
            # j descending inside each group (diagonal/masked block first)
            chunks = []
            seq_off = 0
            for nblk in seq_blocks:
                for g in range(nblk):
                    js = list(range(g, -1, -1))
                    for i0 in range(0, len(js), CH):
                        chunks.append((seq_off, nblk, g, js[i0:i0 + CH]))
                seq_off += nblk * BLK

            state = {}    # (seq_off, g) -> (oT_ps, acc or None)
            pending = []

            def emit_front(ch):
                seq_off, nblk, g, js = ch
                nj = len(js)
                s3 = ps_s.tile([BLK, CH, H, BLK], f32, tag="s3", name="s3")
                qg = q_sl(seq_off, g)
                for jj, j in enumerate(js):
                    nc.tensor.matmul(
                        s3[:, jj], k_sl(seq_off, j), qg,
                        start=True, stop=True,
                    )
                p3 = p_pool.tile([BLK, CH, H, BLK], bf16, tag="p3", name="p3")
                nc.scalar.activation(
                    p3[:, :nj], s3[:, :nj],
                    mybir.ActivationFunctionType.Exp,
                    scale=SCALE,
                )
                if js[0] == g:  # diagonal: zero the upper triangle
                    nc.gpsimd.tensor_mul(p3[:, 0], p3[:, 0], mask_sb[:])
                return s3, p3

            def emit_back(ch, s3, p3):
                seq_off, nblk, g, js = ch
                Q0 = seq_off + g * BLK
                key = (seq_off, g)
                first = js[0] == g
                last = js[-1] == 0
                b = g + 1
                use_acc = b >= ACC_MIN
                if first:
                    oT_ps = ps_o.tile([BLK, H, BLK], f32, tag="ot",
                                      name="oT_ps")
                    acc = None
                    if use_acc:
                        acc = acc_pool.tile([BLK, H, BLK], bf16, tag="acc",
                                            name="acc")
                    state[key] = (oT_ps, acc)
                oT_ps, acc = state[key]
                for jj, j in enumerate(js):
                    nc.tensor.matmul(
                        oT_ps[:], v_sl(seq_off, j), p3[:, jj],
                        start=(j == g), stop=(j == 0),
                    )
                # softmax denominators land in this chunk's s3 slot 0 --
                # the scores there are dead once exp has read them, so no
                # separate PSUM pool is needed (saves 2 banks -> oT can
                # double-buffer)
                sums_ps = s3[:, 0]
                if use_acc:
                    # flat contiguous [128, 512] APs so the DVE engages
                    # its 2x bf16 packed mode
                    af = acc.rearrange("p h q -> p (h q)")

                    def pf(jj):
                        return p3[:, jj].rearrange("p h q -> p (h q)")

                    if first:
                        # first add on GpSimd (it has slack); rest on DVE
                        nc.gpsimd.tensor_add(af, pf(0), pf(1))
                        for jj in range(2, len(js)):
                            nc.vector.tensor_add(af, af, pf(jj))
                    else:
                        for jj in range(len(js)):
                            nc.vector.tensor_add(af, af, pf(jj))
                    if last:
                        nc.tensor.matmul(sums_ps, ones_b[:], acc[:],
                                         start=True, stop=True)
                else:
                    for jj, j in enumerate(js):
                        nc.tensor.matmul(
                            sums_ps, ones_b[:], p3[:, jj],
                            start=(j == g), stop=(j == 0),
                        )

                if last:
                    nc.vector.tensor_copy(
                        ot_stage[:, :, Q0:Q0 + BLK], oT_ps[:]
                    )
                    nc.vector.tensor_copy(
                        sums_stage[:, :, Q0:Q0 + BLK], sums_ps[0:1]
                    )
                    del state[key]
                    # stream outputs: ~4-group slabs, per-group on the
                    # final sequence so the drain tail stays short
                    last_seq = seq_off + nblk * BLK == T
                    slab = 1 if last_seq else 4
                    if (g + 1) % slab == 0 or g == nblk - 1:
                        lo = seq_off + (g - (g % slab)) * BLK
                        if g == nblk - 1 and (g + 1) % slab != 0:
                            lo = seq_off + (g + 1 - ((g + 1) % slab)) * BLK
                        hi = Q0 + BLK
                        nc.sync.dma_start(oT_ext[:, :, lo:hi],
                                          ot_stage[:, :, lo:hi])
                    if g == nblk - 1:
                        nc.sync.dma_start(
                            sums_ext[:, :, seq_off:seq_off + nblk * BLK],
                            sums_stage[:, :, seq_off:seq_off + nblk * BLK],
                        )

            for ch in chunks:
                s3, p3 = emit_front(ch)
                pending.append((ch, s3, p3))
                if len(pending) > LAG:
                    emit_back(*pending.pop(0))
            for ch, s3, p3 in pending:
                emit_back(ch, s3, p3)

    nc.finalize()
    return nc


def _install_ntff_hook():
    """Shim antenv.axon_hooks (absent in this container) so trace=True can
    reach the terminal's NRT profiler via libaxon_pjrt.so ctypes."""
    import types

    if "antenv.axon_hooks" in sys.modules:
        return
    import antenv
    from concourse import bass_utils

    mod = types.ModuleType("antenv.axon_hooks")
    state = {"hook": None}
    mod.set_axon_ntff_profile_hook = lambda h: state.__setitem__("hook", h)
    mod.get_axon_ntff_profile_hook = lambda: state["hook"]
    sys.modules["antenv.axon_hooks"] = mod
    antenv.axon_hooks = mod
    bass_utils.upload_artifacts = lambda tmpdir: tmpdir  # zero-egress container
    try:
        if "/root/.axon_site" not in sys.path:
            sys.path.insert(0, "/root/.axon_site")
        from trn_agent_boot.trn_boot import _ntff_profile_via_ctypes

        mod.set_axon_ntff_profile_hook(
            _ntff_profile_via_ctypes("/opt/axon/libaxon_pjrt.so")
        )
    except Exception:
        pass


def kernel(q, k, v, cu_seqlens, max_seqlen):
    from concourse import bass_utils

    q = np.asarray(q, dtype=np.float32)
    k = np.asarray(k, dtype=np.float32)
    v = np.asarray(v, dtype=np.float32)
    cu = np.asarray(cu_seqlens, dtype=np.int64)
    T_host = q.shape[0]
    lengths = np.diff(cu).astype(np.int64)
    all_nblocks = [int((L + BLK - 1) // BLK) for L in lengths]
    T_pad = sum(all_nblocks) * BLK

    # process sequences longest-first: big seq warms the pipeline while the
    # rest of the data streams in, and the tail drains a small seq whose
    # back-phase work (AV/sums per group) is minimal
    order = sorted(range(len(lengths)), key=lambda s: -all_nblocks[s])
    nblocks = [all_nblocks[s] for s in order]

    # host -> padded device token index map (valid tokens only)
    dev_idx = np.zeros(T_host, dtype=np.int64)
    pad_off = 0
    for s in order:
        L = int(lengths[s])
        dev_idx[cu[s]:cu[s] + L] = pad_off + np.arange(L)
        pad_off += all_nblocks[s] * BLK

    bf16 = ml_dtypes.bfloat16
    qp = np.zeros((T_pad, NUM_HEADS * HEAD_DIM), bf16)
    kp = np.zeros((T_pad, NUM_KV_HEADS * HEAD_DIM), bf16)
    vp = np.zeros((T_pad, NUM_KV_HEADS * HEAD_DIM), bf16)
    qp[dev_idx] = q.astype(bf16)
    kp[dev_idx] = k.astype(bf16)
    vp[dev_idx] = v.astype(bf16)

    mask1 = np.where(
        np.arange(BLK)[:, None] <= np.arange(BLK)[None, :], 1.0, 0.0
    ).astype(bf16)
    mask = np.broadcast_to(
        mask1[:, None, :], (BLK, HEADS_PER_CORE, BLK)
    ).copy()

    key = tuple(nblocks)
    if key not in _GRAPH_CACHE:
        _GRAPH_CACHE[key] = _build_graph(key)
    nc = _GRAPH_CACHE[key]

    in_maps = []
    for c in range(N_CORES):
        m = {"mask": mask}
        m["kT"] = np.ascontiguousarray(kp[:, c * HEAD_DIM:(c + 1) * HEAD_DIM].T)
        m["v"] = np.ascontiguousarray(vp[:, c * HEAD_DIM:(c + 1) * HEAD_DIM])
        # [d, h, t] head-interleaved Q^T so all 4 heads ride one 3D AP
        qc = qp[:, c * HEADS_PER_CORE * HEAD_DIM:(c + 1) * HEADS_PER_CORE * HEAD_DIM]
        m["qT"] = np.ascontiguousarray(
            qc.reshape(T_pad, HEADS_PER_CORE, HEAD_DIM).transpose(2, 1, 0)
        )
        in_maps.append(m)

    trace = bool(os.environ.get("BASS_TRACE"))
    if trace:
        _install_ntff_hook()
    res = bass_utils.run_bass_kernel_spmd(
        nc, in_maps, core_ids=list(range(N_CORES)), trace=trace
    )
    if trace and res.exec_time_ns is not None:
        print(f"HW exec time: {res.exec_time_ns} ns")
        if res.instructions_and_trace is not None:
            print(f"trace: {res.instructions_and_trace[1]}")

    out = np.empty((T_host, NUM_HEADS * HEAD_DIM), np.float32)
    for c in range(N_CORES):
        r = res.results[c]
        oT = np.asarray(r["oT"], dtype=np.float32)  # [128, H, T_pad] bf16
        sums = np.asarray(r["sums"], dtype=np.float32)[0]  # [H, T_pad]
        for h in range(HEADS_PER_CORE):
            gh = c * HEADS_PER_CORE + h
            o = (oT[:, h][:, dev_idx] / sums[h][dev_idx][None, :]).T
            out[:, gh * HEAD_DIM:(gh + 1) * HEAD_DIM] = o
    return out


# revision 15
# speedup vs baseline: 1.2835x; 1.2835x over previous
"""Varlen causal GQA attention on 8 TRN2 NeuronCores.

Problem: 32 q heads, 8 kv heads, head_dim 128, ragged batch (cu_seqlens),
f32. Sharded by KV-head group: core c owns kv head c and q heads
4c..4c+3 -- fully data-independent across cores, no collectives.

Per core, blockwise causal attention in 128x128 blocks with all 4 q
heads fused through 3D access patterns (q stored head-interleaved
[d, h, t]). Work per 128-col k-block j against q-block g:
    S[k, h, q]  = (K_j)^T.T @ Q^T      one matmul per (g, j) block
    P = exp(S * scale)                 ONE ScalarE exp per chunk of
                                       CH=3 j-blocks (ACT is the
                                       critical engine: ~51us floor)
    causal mask: 0/1 multiply on GpSimd (diagonal blocks only)
    O^T[h] += V_j @ P                  one matmul, PSUM-accumulated
    softmax denominators: mixed strategy balancing PE vs DVE:
      small groups (<= ACC_MIN blocks): ones^T @ P per block (PE)
      large groups: P accumulated across j on DVE/GpSimd in bf16,
        then ONE ones^T @ acc matmul per group
Engine budget per core (95 blocks): ACT 41 exp instrs ~51us,
PE ~215 matmuls ~50us, DVE adds+PSUM-evac copies ~49us, GpSimd
masks+first-adds ~47us -- all co-binding near the ACT floor.

The instruction stream is software-pipelined at ISSUE level: AV/sums
trail their S/exp by LAG chunks so the PE FIFO never parks on an exp
wait. Scalar ring issues NO DMAs (exp only); all input DMAs ride the
Sync ring in 3 tiers (first 2 blocks -> rest of seq 0 -> per-seq bulk)
so the first matmul starts ~2us in; outputs stream out per ~4-group
slab (per-group for the last seq) to keep the drain tail short. Host
does all transposes (Q^T/K^T in, O^T -> O out), bf16 conversion,
padding of ragged sequences to 128 multiples, and the final softmax
division. oT is returned in bf16 (halves output DMA bytes).
"""

import math
import os
import sys

sys.path.insert(0, "/opt/trn_rl_repo")

import ml_dtypes
import numpy as np

NUM_HEADS = 32
NUM_KV_HEADS = 8
HEAD_DIM = 128
HEADS_PER_CORE = NUM_HEADS // NUM_KV_HEADS  # 4
N_CORES = 8
BLK = 128
SCALE = 1.0 / math.sqrt(HEAD_DIM)

CH = 2        # j-blocks per chunk (PSUM: 2 banks x 2 bufs + oT 2 + sums 2)
ACC_MIN = 4   # groups with >= this many blocks use DVE-accumulated sums
LAG = 3       # chunks the back phase (AV/sums) trails the front (S/exp)
WARM = 6      # dummy matmuls to hold the PE activity monitor at 2.4 GHz
T0_BLOCKS = 2  # tier-0: first blocks of seq 0, smallest possible first DMA

_GRAPH_CACHE = {}


def _build_graph(seq_blocks):
    """Build the SPMD Bacc graph for padded per-seq block counts."""
    from concourse import bacc
    import concourse.mybir as mybir
    from concourse.tile import TileContext

    f32 = mybir.dt.float32
    bf16 = mybir.dt.bfloat16
    T = sum(seq_blocks) * BLK
    n_blocks_total = T // BLK
    H = HEADS_PER_CORE

    nc = bacc.Bacc("TRN2", target_bir_lowering=False, debug=False,
                   num_devices=N_CORES)

    qT_ext = nc.declare_dram_parameter("qT", [BLK, H, T], bf16, isOutput=False)
    kT_ext = nc.declare_dram_parameter("kT", [BLK, T], bf16, isOutput=False)
    v_ext = nc.declare_dram_parameter("v", [T, HEAD_DIM], bf16, isOutput=False)
    mask_ext = nc.declare_dram_parameter("mask", [BLK, H, BLK], bf16,
                                         isOutput=False)
    oT_ext = nc.declare_dram_parameter("oT", [BLK, H, T], bf16, isOutput=True)
    sums_ext = nc.declare_dram_parameter("sums", [1, H, T], f32, isOutput=True)

    nb0 = seq_blocks[0]
    t0 = min(T0_BLOCKS, nb0)  # tier-0 block count
    c_t0 = t0 * BLK
    c_s0 = nb0 * BLK

    with TileContext(nc) as tc:
        with (
            tc.tile_pool(name="persist", bufs=1) as persist,
            tc.tile_pool(name="p", bufs=7) as p_pool,
            tc.tile_pool(name="accp", bufs=2) as acc_pool,
            tc.tile_pool(name="ps_s", bufs=2, space="PSUM") as ps_s,
            tc.tile_pool(name="ps_o", bufs=2, space="PSUM") as ps_o,
            tc.tile_pool(name="ps_sum", bufs=2, space="PSUM") as ps_sum,
        ):
            v_re = v_ext[:].rearrange("(j p) d -> p j d", p=BLK)

            # tier-0: the opening blocks of seq 0 in their own small tiles
            # (dependencies are tile-granular; these land in ~1.5us so the
            # first real matmul isn't gated on bulk transfers)
            kT_q0 = persist.tile([BLK, c_t0], bf16)
            qT_q0 = persist.tile([BLK, H, c_t0], bf16)
            v_q0 = persist.tile([BLK, t0, HEAD_DIM], bf16)
            mask_sb = persist.tile([BLK, H, BLK], bf16)
            nc.sync.dma_start(kT_q0[:], kT_ext[:, :c_t0])
            nc.sync.dma_start(qT_q0[:], qT_ext[:, :, :c_t0])
            nc.sync.dma_start(v_q0[:], v_re[:, :t0, :])
            nc.sync.dma_start(mask_sb[:], mask_ext[:])

            # tier-1: the rest of seq 0, issued from the Scalar ring (its
            # HWDGE queue is a second independent DMA ring; the exp stream
            # hasn't started yet so these issues stall nothing)
            kT_q1 = persist.tile([BLK, c_s0 - c_t0], bf16)
            qT_q1 = persist.tile([BLK, H, c_s0 - c_t0], bf16)
            v_q1 = persist.tile([BLK, nb0 - t0, HEAD_DIM], bf16)
            if nb0 > t0:
                nc.scalar.dma_start(kT_q1[:], kT_ext[:, c_t0:c_s0])
                nc.scalar.dma_start(v_q1[:], v_re[:, t0:nb0, :])
                nc.scalar.dma_start(qT_q1[:], qT_ext[:, :, c_t0:c_s0])

            # tier-2: remaining sequences, one set of DMAs per sequence,
            # spread across three DMA rings (sync HWDGE / scalar HWDGE /
            # gpsimd SWDGE) so no single ring's ~155 GB/s bounds the input
            kT_sb = persist.tile([BLK, max(T - c_s0, BLK)], bf16)
            qT_sb = persist.tile([BLK, H, max(T - c_s0, BLK)], bf16)
            v_sb = persist.tile(
                [BLK, max(n_blocks_total - nb0, 1), HEAD_DIM], bf16)
            off = c_s0
            for si, nblk in enumerate(seq_blocks[1:]):
                o0 = off - c_s0
                j0 = o0 // BLK
                ls = nblk * BLK
                kv_eng = nc.sync if si == 0 else nc.gpsimd
                q_eng = nc.sync if si == 0 else nc.scalar
                kv_eng.dma_start(kT_sb[:, o0:o0 + ls],
                                 kT_ext[:, off:off + ls])
                kv_eng.dma_start(v_sb[:, j0:j0 + nblk, :],
                                 v_re[:, off // BLK:off // BLK + nblk, :])
                q_eng.dma_start(qT_sb[:, :, o0:o0 + ls],
                                qT_ext[:, :, off:off + ls])
                off += ls

            ones_f = persist.tile([BLK, BLK], f32)
            nc.vector.memset(ones_f[:], 1.0)
            # full [128,128] ones stationary: sums matmul runs M=128 so the
            # PE array never reconfigures col groups between AV and sums
            ones_b = persist.tile([BLK, BLK], bf16)
            nc.vector.tensor_copy(ones_b[:], ones_f[:])

            ot_stage = persist.tile([BLK, H, T], bf16)
            sums_stage = persist.tile([1, H, T], f32)

            # HAM warm-up: dummy matmuls (no DMA deps) cover the tier-0 DMA
            # latency and keep the PE activity monitor at full clock
            warm_sb = persist.tile([BLK, 4 * BLK], bf16)
            nc.vector.memset(warm_sb[:], 1.0)
            for _ in range(WARM):
                warm_ps = ps_s.tile([BLK, CH, H, BLK], f32, tag="s3",
                                    name="warm")
                nc.tensor.matmul(
                    warm_ps[:, 0].rearrange("p h q -> p (h q)"),
                    warm_sb[:, :BLK], warm_sb[:],
                    start=True, stop=True,
                )

            def k_sl(seq_off, j):
                if seq_off == 0:
                    if j < t0:
                        return kT_q0[:, j * BLK:(j + 1) * BLK]
                    return kT_q1[:, (j - t0) * BLK:(j - t0 + 1) * BLK]
                c = seq_off - c_s0 + j * BLK
                return kT_sb[:, c:c + BLK]

            def q_sl(seq_off, g):
                if seq_off == 0:
                    if g < t0:
                        return qT_q0[:, :, g * BLK:(g + 1) * BLK]
                    return qT_q1[:, :, (g - t0) * BLK:(g - t0 + 1) * BLK]
                c = seq_off - c_s0 + g * BLK
                return qT_sb[:, :, c:c + BLK]

            def v_sl(seq_off, j):
                if seq_off == 0:
                    if j < t0:
                        return v_q0[:, j, :]
                    return v_q1[:, j - t0, :]
                return v_sb[:, (seq_off - c_s0) // BLK + j, :]

            # flat chunk stream over (seq, q-block g, k-block j-triples),
            # j descending inside each group (diagonal/masked block first)
            chunks = []
            seq_off = 0
            for nblk in seq_blocks:
                for g in range(nblk):
                    js = list(range(g, -1, -1))
                    for i0 in range(0, len(js), CH):
                        chunks.append((seq_off, nblk, g, js[i0:i0 + CH]))
                seq_off += nblk * BLK

            state = {}    # (seq_off, g) -> (oT_ps, acc or None)
            pending = []

            def emit_front(ch):
                seq_off, nblk, g, js = ch
                nj = len(js)
                s3 = ps_s.tile([BLK, CH, H, BLK], f32, tag="s3", name="s3")
                qg = q_sl(seq_off, g)
                for jj, j in enumerate(js):
                    nc.tensor.matmul(
                        s3[:, jj], k_sl(seq_off, j), qg,
                        start=True, stop=True,
                    )
                p3 = p_pool.tile([BLK, CH, H, BLK], bf16, tag="p3", name="p3")
                nc.scalar.activation(
                    p3[:, :nj], s3[:, :nj],
                    mybir.ActivationFunctionType.Exp,
                    scale=SCALE,
                )
                if js[0] == g:  # diagonal: zero the upper triangle
                    nc.gpsimd.tensor_mul(p3[:, 0], p3[:, 0], mask_sb[:])
                return s3, p3

            def emit_back(ch, s3, p3):
                seq_off, nblk, g, js = ch
                Q0 = seq_off + g * BLK
                key = (seq_off, g)
                first = js[0] == g
                last = js[-1] == 0
                b = g + 1
                use_acc = b >= ACC_MIN
                if first:
                    oT_ps = ps_o.tile([BLK, H, BLK], f32, tag="ot",
                                      name="oT_ps")
                    acc = None
                    if use_acc:
                        acc = acc_pool.tile([BLK, H, BLK], bf16, tag="acc",
                                            name="acc")
                    state[key] = {"ot": oT_ps, "acc": acc, "sums": None}
                st = state[key]
                oT_ps, acc = st["ot"], st["acc"]
                for jj, j in enumerate(js):
                    nc.tensor.matmul(
                        oT_ps[:], v_sl(seq_off, j), p3[:, jj],
                        start=(j == g), stop=(j == 0),
                    )
                if use_acc:
                    # flat contiguous [128, 512] APs so the DVE engages
                    # its 2x bf16 packed mode
                    af = acc.rearrange("p h q -> p (h q)")

                    def pf(jj):
                        return p3[:, jj].rearrange("p h q -> p (h q)")

                    if first:
                        # first add on GpSimd (it has slack); rest on DVE
                        nc.gpsimd.tensor_add(af, pf(0), pf(1))
                        for jj in range(2, len(js)):
                            nc.vector.tensor_add(af, af, pf(jj))
                    else:
                        for jj in range(len(js)):
                            nc.vector.tensor_add(af, af, pf(jj))
                    if last:
                        sums_ps = ps_sum.tile([BLK, H, BLK], f32,
                                              tag="sums", name="sums_ps")
                        st["sums"] = sums_ps
                        nc.tensor.matmul(sums_ps[:], ones_b[:], acc[:],
                                         start=True, stop=True)
                else:
                    if first:
                        st["sums"] = ps_sum.tile([BLK, H, BLK], f32,
                                                 tag="sums", name="sums_ps")
                    sums_ps = st["sums"]
                    for jj, j in enumerate(js):
                        nc.tensor.matmul(
                            sums_ps[:], ones_b[:], p3[:, jj],
                            start=(j == g), stop=(j == 0),
                        )

                if last:
                    sums_ps = st["sums"]
                    nc.vector.tensor_copy(
                        ot_stage[:, :, Q0:Q0 + BLK], oT_ps[:]
                    )
                    nc.vector.tensor_copy(
                        sums_stage[:, :, Q0:Q0 + BLK], sums_ps[0:1]
                    )
                    del state[key]
                    # stream outputs: ~4-group slabs, per-group on the
                    # final sequence so the drain tail stays short
                    last_seq = seq_off + nblk * BLK == T
                    slab = 1 if last_seq else 4
                    if (g + 1) % slab == 0 or g == nblk - 1:
                        lo = seq_off + (g - (g % slab)) * BLK
                        if g == nblk - 1 and (g + 1) % slab != 0:
                            lo = seq_off + (g + 1 - ((g + 1) % slab)) * BLK
                        hi = Q0 + BLK
                        nc.sync.dma_start(oT_ext[:, :, lo:hi],
                                          ot_stage[:, :, lo:hi])
                    if g == nblk - 1:
                        nc.sync.dma_start(
                            sums_ext[:, :, seq_off:seq_off + nblk * BLK],
                            sums_stage[:, :, seq_off:seq_off + nblk * BLK],
                        )

            for ch in chunks:
                s3, p3 = emit_front(ch)
                pending.append((ch, s3, p3))
                if len(pending) > LAG:
                    emit_back(*pending.pop(0))
            for ch, s3, p3 in pending:
                emit_back(ch, s3, p3)

    nc.finalize()
    return nc


def _install_ntff_hook():
    """Shim antenv.axon_hooks (absent in this container) so trace=True can
    reach the terminal's NRT profiler via libaxon_pjrt.so ctypes."""
    import types

    if "antenv.axon_hooks" in sys.modules:
        return
    import antenv
    from concourse import bass_utils

    mod = types.ModuleType("antenv.axon_hooks")
    state = {"hook": None}
    mod.set_axon_ntff_profile_hook = lambda h: state.__setitem__("hook", h)
    mod.get_axon_ntff_profile_hook = lambda: state["hook"]
    sys.modules["antenv.axon_hooks"] = mod
    antenv.axon_hooks = mod
    bass_utils.upload_artifacts = lambda tmpdir: tmpdir  # zero-egress container
    try:
        if "/root/.axon_site" not in sys.path:
            sys.path.insert(0, "/root/.axon_site")
        from trn_agent_boot.trn_boot import _ntff_profile_via_ctypes

        mod.set_axon_ntff_profile_hook(
            _ntff_profile_via_ctypes("/opt/axon/libaxon_pjrt.so")
        )
    except Exception:
        pass


def kernel(q, k, v, cu_seqlens, max_seqlen):
    from concourse import bass_utils

    q = np.asarray(q, dtype=np.float32)
    k = np.asarray(k, dtype=np.float32)
    v = np.asarray(v, dtype=np.float32)
    cu = np.asarray(cu_seqlens, dtype=np.int64)
    T_host = q.shape[0]
    lengths = np.diff(cu).astype(np.int64)
    all_nblocks = [int((L + BLK - 1) // BLK) for L in lengths]
    T_pad = sum(all_nblocks) * BLK

    # process sequences longest-first: big seq warms the pipeline while the
    # rest of the data streams in, and the tail drains a small seq whose
    # back-phase work (AV/sums per group) is minimal
    order = sorted(range(len(lengths)), key=lambda s: -all_nblocks[s])
    nblocks = [all_nblocks[s] for s in order]

    # host -> padded device token index map (valid tokens only)
    dev_idx = np.zeros(T_host, dtype=np.int64)
    pad_off = 0
    for s in order:
        L = int(lengths[s])
        dev_idx[cu[s]:cu[s] + L] = pad_off + np.arange(L)
        pad_off += all_nblocks[s] * BLK

    bf16 = ml_dtypes.bfloat16
    qp = np.zeros((T_pad, NUM_HEADS * HEAD_DIM), bf16)
    kp = np.zeros((T_pad, NUM_KV_HEADS * HEAD_DIM), bf16)
    vp = np.zeros((T_pad, NUM_KV_HEADS * HEAD_DIM), bf16)
    qp[dev_idx] = q.astype(bf16)
    kp[dev_idx] = k.astype(bf16)
    vp[dev_idx] = v.astype(bf16)

    mask1 = np.where(
        np.arange(BLK)[:, None] <= np.arange(BLK)[None, :], 1.0, 0.0
    ).astype(bf16)
    mask = np.broadcast_to(
        mask1[:, None, :], (BLK, HEADS_PER_CORE, BLK)
    ).copy()

    key = tuple(nblocks)
    if key not in _GRAPH_CACHE:
        _GRAPH_CACHE[key] = _build_graph(key)
    nc = _GRAPH_CACHE[key]

    in_maps = []
    for c in range(N_CORES):
        m = {"mask": mask}
        m["kT"] = np.ascontiguousarray(kp[:, c * HEAD_DIM:(c + 1) * HEAD_DIM].T)
        m["v"] = np.ascontiguousarray(vp[:, c * HEAD_DIM:(c + 1) * HEAD_DIM])
        # [d, h, t] head-interleaved Q^T so all 4 heads ride one 3D AP
        qc = qp[:, c * HEADS_PER_CORE * HEAD_DIM:(c + 1) * HEADS_PER_CORE * HEAD_DIM]
        m["qT"] = np.ascontiguousarray(
            qc.reshape(T_pad, HEADS_PER_CORE, HEAD_DIM).transpose(2, 1, 0)
        )
        in_maps.append(m)

    trace = bool(os.environ.get("BASS_TRACE"))
    if trace:
        _install_ntff_hook()
    res = bass_utils.run_bass_kernel_spmd(
        nc, in_maps, core_ids=list(range(N_CORES)), trace=trace
    )
    if trace and res.exec_time_ns is not None:
        print(f"HW exec time: {res.exec_time_ns} ns")
        if res.instructions_and_trace is not None:
            print(f"trace: {res.instructions_and_trace[1]}")

    out = np.empty((T_host, NUM_HEADS * HEAD_DIM), np.float32)
    for c in range(N_CORES):
        r = res.results[c]
        oT = np.asarray(r["oT"], dtype=np.float32)  # [128, H, T_pad] bf16
        sums = np.asarray(r["sums"], dtype=np.float32)[0]  # [H, T_pad]
        for h in range(HEADS_PER_CORE):
            gh = c * HEADS_PER_CORE + h
            o = (oT[:, h][:, dev_idx] / sums[h][dev_idx][None, :]).T
            out[:, gh * HEAD_DIM:(gh + 1) * HEAD_DIM] = o
    return out


# revision 17
# speedup vs baseline: 1.5113x; 1.1775x over previous
"""Varlen causal GQA attention on 8 TRN2 NeuronCores.

Problem: 32 q heads, 8 kv heads, head_dim 128, ragged batch (cu_seqlens),
f32. Sharded by KV-head group: core c owns kv head c and q heads
4c..4c+3 -- fully data-independent across cores, no collectives.

Per core, blockwise causal attention in 128x128 blocks with all 4 q
heads fused through 3D access patterns (q stored head-interleaved
[d, h, t]). Work per 128-col k-block j against q-block g:
    S[k, h, q]  = (K_j)^T.T @ Q^T      one matmul per (g, j) block
    P = exp(S * scale)                 ONE ScalarE exp per chunk of
                                       CH=3 j-blocks (ACT is the
                                       critical engine: ~51us floor)
    causal mask: 0/1 multiply on GpSimd (diagonal blocks only)
    O^T[h] += V_j @ P                  one matmul, PSUM-accumulated
    softmax denominators: mixed strategy balancing PE vs DVE:
      small groups (<= ACC_MIN blocks): ones^T @ P per block (PE)
      large groups: P accumulated across j on DVE/GpSimd in bf16,
        then ONE ones^T @ acc matmul per group
Engine budget per core (95 blocks): ACT 41 exp instrs ~51us,
PE ~215 matmuls ~50us, DVE adds+PSUM-evac copies ~49us, GpSimd
masks+first-adds ~47us -- all co-binding near the ACT floor.

The instruction stream is software-pipelined at ISSUE level: AV/sums
trail their S/exp by LAG chunks so the PE FIFO never parks on an exp
wait. Scalar ring issues NO DMAs (exp only); all input DMAs ride the
Sync ring in 3 tiers (first 2 blocks -> rest of seq 0 -> per-seq bulk)
so the first matmul starts ~2us in; outputs stream out per ~4-group
slab (per-group for the last seq) to keep the drain tail short. Host
does all transposes (Q^T/K^T in, O^T -> O out), bf16 conversion,
padding of ragged sequences to 128 multiples, and the final softmax
division. oT is returned in bf16 (halves output DMA bytes).
"""

import math
import os
import sys

sys.path.insert(0, "/opt/trn_rl_repo")

import ml_dtypes
import numpy as np

NUM_HEADS = 32
NUM_KV_HEADS = 8
HEAD_DIM = 128
HEADS_PER_CORE = NUM_HEADS // NUM_KV_HEADS  # 4
N_CORES = 8
BLK = 128
SCALE = 1.0 / math.sqrt(HEAD_DIM)

CH = 2        # j-blocks per chunk (PSUM: 2 banks x 2 bufs + oT 2 + sums 2)
ACC_MIN = 4   # groups with >= this many blocks use DVE-accumulated sums
LAG = 3       # chunks the back phase (AV/sums) trails the front (S/exp)
WARM = 6      # dummy matmuls to hold the PE activity monitor at 2.4 GHz
T0_BLOCKS = 2  # tier-0: first blocks of seq 0, smallest possible first DMA

_GRAPH_CACHE = {}


def _build_graph(seq_blocks):
    """Build the SPMD Bacc graph for padded per-seq block counts."""
    from concourse import bacc
    import concourse.mybir as mybir
    from concourse.tile import TileContext

    f32 = mybir.dt.float32
    bf16 = mybir.dt.bfloat16
    T = sum(seq_blocks) * BLK
    n_blocks_total = T // BLK
    H = HEADS_PER_CORE

    nc = bacc.Bacc("TRN2", target_bir_lowering=False, debug=False,
                   num_devices=N_CORES)

    qT_ext = nc.declare_dram_parameter("qT", [BLK, H, T], bf16, isOutput=False)
    kT_ext = nc.declare_dram_parameter("kT", [BLK, T], bf16, isOutput=False)
    v_ext = nc.declare_dram_parameter("v", [T, HEAD_DIM], bf16, isOutput=False)
    bias_ext = nc.declare_dram_parameter("bias", [BLK, H, BLK], bf16,
                                         isOutput=False)
    ident_ext = nc.declare_dram_parameter("ident", [BLK, BLK], bf16,
                                          isOutput=False)
    oT_ext = nc.declare_dram_parameter("oT", [BLK, H, T], bf16, isOutput=True)
    sums_ext = nc.declare_dram_parameter("sums", [1, H, T], f32, isOutput=True)

    nb0 = seq_blocks[0]
    t0 = min(T0_BLOCKS, nb0)  # tier-0 block count
    c_t0 = t0 * BLK
    c_s0 = nb0 * BLK

    with TileContext(nc) as tc:
        with (
            tc.tile_pool(name="persist", bufs=1) as persist,
            tc.tile_pool(name="p", bufs=7) as p_pool,
            tc.tile_pool(name="accp", bufs=2) as acc_pool,
            tc.tile_pool(name="ps_s", bufs=2, space="PSUM") as ps_s,
            tc.tile_pool(name="ps_o", bufs=2, space="PSUM") as ps_o,
            tc.tile_pool(name="ps_sum", bufs=2, space="PSUM") as ps_sum,
        ):
            v_re = v_ext[:].rearrange("(j p) d -> p j d", p=BLK)

            # seq 0 arrives in 2-block pair tiles alternating between the
            # two HWDGE rings (sync=Q1, scalar=Q10) ordered by when compute
            # needs each pair; deps are tile-granular so early chunks only
            # wait on their own pair's small transfer
            bias_sb = persist.tile([BLK, H, BLK], bf16)
            ident_sb = persist.tile([BLK, BLK], bf16)
            nc.sync.dma_start(bias_sb[:], bias_ext[:])
            nc.sync.dma_start(ident_sb[:], ident_ext[:])
            npair = (nb0 + 1) // 2
            kT_tl, qT_tl, v_tl = [], [], []
            for p in range(npair):
                lo = p * 2
                hi = min(lo + 2, nb0)
                nb = hi - lo
                kt = persist.tile([BLK, nb * BLK], bf16, name=f"kT_p{p}")
                qt = persist.tile([BLK, H, nb * BLK], bf16, name=f"qT_p{p}")
                vt = persist.tile([BLK, nb, HEAD_DIM], bf16, name=f"v_p{p}")
                kT_tl.append(kt)
                qT_tl.append(qt)
                v_tl.append(vt)
                eng = nc.sync if (p == 0 or p >= 3) else nc.scalar
                eng.dma_start(kt[:], kT_ext[:, lo * BLK:hi * BLK])
                eng.dma_start(qt[:], qT_ext[:, :, lo * BLK:hi * BLK])
                eng.dma_start(vt[:], v_re[:, lo:hi, :])

            # tier-2: remaining sequences, one set of DMAs per sequence,
            # spread across three DMA rings (sync HWDGE / scalar HWDGE /
            # gpsimd SWDGE) so no single ring's ~155 GB/s bounds the input
            kT_sb = persist.tile([BLK, max(T - c_s0, BLK)], bf16)
            qT_sb = persist.tile([BLK, H, max(T - c_s0, BLK)], bf16)
            v_sb = persist.tile(
                [BLK, max(n_blocks_total - nb0, 1), HEAD_DIM], bf16)
            off = c_s0
            for si, nblk in enumerate(seq_blocks[1:]):
                o0 = off - c_s0
                j0 = o0 // BLK
                ls = nblk * BLK
                kv_eng = nc.sync if si == 0 else nc.gpsimd
                q_eng = nc.sync if si == 0 else nc.scalar
                kv_eng.dma_start(kT_sb[:, o0:o0 + ls],
                                 kT_ext[:, off:off + ls])
                kv_eng.dma_start(v_sb[:, j0:j0 + nblk, :],
                                 v_re[:, off // BLK:off // BLK + nblk, :])
                q_eng.dma_start(qT_sb[:, :, o0:o0 + ls],
                                qT_ext[:, :, off:off + ls])
                off += ls

            ones_f = persist.tile([BLK, BLK], f32)
            nc.vector.memset(ones_f[:], 1.0)
            # full [128,128] ones stationary: sums matmul runs M=128 so the
            # PE array never reconfigures col groups between AV and sums
            ones_b = persist.tile([BLK, BLK], bf16)
            nc.vector.tensor_copy(ones_b[:], ones_f[:])

            ot_stage = persist.tile([BLK, H, T], bf16)
            sums_stage = persist.tile([1, H, T], f32)

            # HAM warm-up: dummy matmuls (no DMA deps) cover the tier-0 DMA
            # latency and keep the PE activity monitor at full clock
            warm_sb = persist.tile([BLK, 4 * BLK], bf16)
            nc.vector.memset(warm_sb[:], 1.0)
            for _ in range(WARM):
                warm_ps = ps_s.tile([BLK, CH, H, BLK], f32, tag="s3",
                                    name="warm")
                nc.tensor.matmul(
                    warm_ps[:, 0].rearrange("p h q -> p (h q)"),
                    warm_sb[:, :BLK], warm_sb[:],
                    start=True, stop=True,
                )

            def k_sl(seq_off, j):
                if seq_off == 0:
                    r = j % 2
                    return kT_tl[j // 2][:, r * BLK:(r + 1) * BLK]
                c = seq_off - c_s0 + j * BLK
                return kT_sb[:, c:c + BLK]

            def q_sl(seq_off, g):
                if seq_off == 0:
                    r = g % 2
                    return qT_tl[g // 2][:, :, r * BLK:(r + 1) * BLK]
                c = seq_off - c_s0 + g * BLK
                return qT_sb[:, :, c:c + BLK]

            def v_sl(seq_off, j):
                if seq_off == 0:
                    return v_tl[j // 2][:, j % 2, :]
                return v_sb[:, (seq_off - c_s0) // BLK + j, :]

            # flat chunk stream over (seq, q-block g, k-block j-triples),
            # j descending inside each group (diagonal/masked block first)
            chunks = []
            seq_off = 0
            for nblk in seq_blocks:
                for g in range(nblk):
                    js = list(range(g, -1, -1))
                    for i0 in range(0, len(js), CH):
                        chunks.append((seq_off, nblk, g, js[i0:i0 + CH]))
                seq_off += nblk * BLK

            state = {}    # (seq_off, g) -> (oT_ps, acc or None)
            pending = []

            def emit_front(ch):
                seq_off, nblk, g, js = ch
                nj = len(js)
                s3 = ps_s.tile([BLK, CH, H, BLK], f32, tag="s3", name="s3")
                qg = q_sl(seq_off, g)
                diag = js[0] == g
                if diag:
                    # pre-load the causal bias into the diag slot's PSUM
                    # bank through the PE (sets has_written) so the S
                    # matmul accumulates onto it and exp() zeroes the
                    # upper triangle -- no elementwise mask op needed
                    nc.tensor.matmul(s3[:, 0], ident_sb[:], bias_sb[:],
                                     start=True, stop=False)
                for jj, j in enumerate(js):
                    nc.tensor.matmul(
                        s3[:, jj], k_sl(seq_off, j), qg,
                        start=not (diag and jj == 0), stop=True,
                    )
                p3 = p_pool.tile([BLK, CH, H, BLK], bf16, tag="p3", name="p3")
                nc.scalar.activation(
                    p3[:, :nj], s3[:, :nj],
                    mybir.ActivationFunctionType.Exp,
                    scale=SCALE,
                )
                return s3, p3

            def emit_back(ch, s3, p3):
                seq_off, nblk, g, js = ch
                Q0 = seq_off + g * BLK
                key = (seq_off, g)
                first = js[0] == g
                last = js[-1] == 0
                b = g + 1
                use_acc = b >= ACC_MIN
                if first:
                    oT_ps = ps_o.tile([BLK, H, BLK], f32, tag="ot",
                                      name="oT_ps")
                    acc = None
                    if use_acc:
                        acc = acc_pool.tile([BLK, H, BLK], bf16, tag="acc",
                                            name="acc")
                    state[key] = {"ot": oT_ps, "acc": acc, "sums": None}
                st = state[key]
                oT_ps, acc = st["ot"], st["acc"]
                for jj, j in enumerate(js):
                    nc.tensor.matmul(
                        oT_ps[:], v_sl(seq_off, j), p3[:, jj],
                        start=(j == g), stop=(j == 0),
                    )
                if use_acc:
                    # flat contiguous [128, 512] APs so the DVE engages
                    # its 2x bf16 packed mode
                    af = acc.rearrange("p h q -> p (h q)")

                    def pf(jj):
                        return p3[:, jj].rearrange("p h q -> p (h q)")

                    if first:
                        # first add on GpSimd (it has slack); rest on DVE
                        nc.gpsimd.tensor_add(af, pf(0), pf(1))
                        for jj in range(2, len(js)):
                            nc.vector.tensor_add(af, af, pf(jj))
                    else:
                        for jj in range(len(js)):
                            nc.vector.tensor_add(af, af, pf(jj))
                    if last:
                        sums_ps = ps_sum.tile([BLK, H, BLK], f32,
                                              tag="sums", name="sums_ps")
                        st["sums"] = sums_ps
                        nc.tensor.matmul(sums_ps[:], ones_b[:], acc[:],
                                         start=True, stop=True)
                else:
                    if first:
                        st["sums"] = ps_sum.tile([BLK, H, BLK], f32,
                                                 tag="sums", name="sums_ps")
                    sums_ps = st["sums"]
                    for jj, j in enumerate(js):
                        nc.tensor.matmul(
                            sums_ps[:], ones_b[:], p3[:, jj],
                            start=(j == g), stop=(j == 0),
                        )

                if last:
                    sums_ps = st["sums"]
                    nc.vector.tensor_copy(
                        ot_stage[:, :, Q0:Q0 + BLK], oT_ps[:]
                    )
                    nc.vector.tensor_copy(
                        sums_stage[:, :, Q0:Q0 + BLK], sums_ps[0:1]
                    )
                    del state[key]
                    # stream outputs: ~4-group slabs, per-group on the
                    # final sequence so the drain tail stays short
                    last_seq = seq_off + nblk * BLK == T
                    slab = 1 if last_seq else 4
                    if (g + 1) % slab == 0 or g == nblk - 1:
                        lo = seq_off + (g - (g % slab)) * BLK
                        if g == nblk - 1 and (g + 1) % slab != 0:
                            lo = seq_off + (g + 1 - ((g + 1) % slab)) * BLK
                        hi = Q0 + BLK
                        nc.sync.dma_start(oT_ext[:, :, lo:hi],
                                          ot_stage[:, :, lo:hi])
                    if g == nblk - 1:
                        nc.sync.dma_start(
                            sums_ext[:, :, seq_off:seq_off + nblk * BLK],
                            sums_stage[:, :, seq_off:seq_off + nblk * BLK],
                        )

            for ch in chunks:
                s3, p3 = emit_front(ch)
                pending.append((ch, s3, p3))
                if len(pending) > LAG:
                    emit_back(*pending.pop(0))
            for ch, s3, p3 in pending:
                emit_back(ch, s3, p3)

    nc.finalize()
    return nc


def _install_ntff_hook():
    """Shim antenv.axon_hooks (absent in this container) so trace=True can
    reach the terminal's NRT profiler via libaxon_pjrt.so ctypes."""
    import types

    if "antenv.axon_hooks" in sys.modules:
        return
    import antenv
    from concourse import bass_utils

    mod = types.ModuleType("antenv.axon_hooks")
    state = {"hook": None}
    mod.set_axon_ntff_profile_hook = lambda h: state.__setitem__("hook", h)
    mod.get_axon_ntff_profile_hook = lambda: state["hook"]
    sys.modules["antenv.axon_hooks"] = mod
    antenv.axon_hooks = mod
    bass_utils.upload_artifacts = lambda tmpdir: tmpdir  # zero-egress container
    try:
        if "/root/.axon_site" not in sys.path:
            sys.path.insert(0, "/root/.axon_site")
        from trn_agent_boot.trn_boot import _ntff_profile_via_ctypes

        mod.set_axon_ntff_profile_hook(
            _ntff_profile_via_ctypes("/opt/axon/libaxon_pjrt.so")
        )
    except Exception:
        pass


def kernel(q, k, v, cu_seqlens, max_seqlen):
    from concourse import bass_utils

    q = np.asarray(q, dtype=np.float32)
    k = np.asarray(k, dtype=np.float32)
    v = np.asarray(v, dtype=np.float32)
    cu = np.asarray(cu_seqlens, dtype=np.int64)
    T_host = q.shape[0]
    lengths = np.diff(cu).astype(np.int64)
    all_nblocks = [int((L + BLK - 1) // BLK) for L in lengths]
    T_pad = sum(all_nblocks) * BLK

    # process sequences longest-first: big seq warms the pipeline while the
    # rest of the data streams in, and the tail drains a small seq whose
    # back-phase work (AV/sums per group) is minimal
    order = sorted(range(len(lengths)), key=lambda s: -all_nblocks[s])
    nblocks = [all_nblocks[s] for s in order]

    # host -> padded device token index map (valid tokens only)
    dev_idx = np.zeros(T_host, dtype=np.int64)
    pad_off = 0
    for s in order:
        L = int(lengths[s])
        dev_idx[cu[s]:cu[s] + L] = pad_off + np.arange(L)
        pad_off += all_nblocks[s] * BLK

    bf16 = ml_dtypes.bfloat16
    qp = np.zeros((T_pad, NUM_HEADS * HEAD_DIM), bf16)
    kp = np.zeros((T_pad, NUM_KV_HEADS * HEAD_DIM), bf16)
    vp = np.zeros((T_pad, NUM_KV_HEADS * HEAD_DIM), bf16)
    qp[dev_idx] = q.astype(bf16)
    kp[dev_idx] = k.astype(bf16)
    vp[dev_idx] = v.astype(bf16)

    # causal-mask bias: the diagonal S block accumulates onto this via a
    # preceding identity matmul, so exp() zeroes the upper triangle with no
    # elementwise mask op anywhere
    bias1 = np.where(
        np.arange(BLK)[:, None] > np.arange(BLK)[None, :], -480.0, 0.0
    ).astype(bf16)
    bias = np.ascontiguousarray(np.broadcast_to(
        bias1[:, None, :], (BLK, HEADS_PER_CORE, BLK)
    ))
    ident = np.eye(BLK, dtype=bf16)

    key = tuple(nblocks)
    if key not in _GRAPH_CACHE:
        _GRAPH_CACHE[key] = _build_graph(key)
    nc = _GRAPH_CACHE[key]

    in_maps = []
    for c in range(N_CORES):
        m = {"bias": bias, "ident": ident}
        m["kT"] = np.ascontiguousarray(kp[:, c * HEAD_DIM:(c + 1) * HEAD_DIM].T)
        m["v"] = np.ascontiguousarray(vp[:, c * HEAD_DIM:(c + 1) * HEAD_DIM])
        # [d, h, t] head-interleaved Q^T so all 4 heads ride one 3D AP
        qc = qp[:, c * HEADS_PER_CORE * HEAD_DIM:(c + 1) * HEADS_PER_CORE * HEAD_DIM]
        m["qT"] = np.ascontiguousarray(
            qc.reshape(T_pad, HEADS_PER_CORE, HEAD_DIM).transpose(2, 1, 0)
        )
        in_maps.append(m)

    trace = bool(os.environ.get("BASS_TRACE"))
    if trace:
        _install_ntff_hook()
    res = bass_utils.run_bass_kernel_spmd(
        nc, in_maps, core_ids=list(range(N_CORES)), trace=trace
    )
    if trace and res.exec_time_ns is not None:
        print(f"HW exec time: {res.exec_time_ns} ns")
        if res.instructions_and_trace is not None:
            print(f"trace: {res.instructions_and_trace[1]}")

    out = np.empty((T_host, NUM_HEADS * HEAD_DIM), np.float32)
    for c in range(N_CORES):
        r = res.results[c]
        oT = np.asarray(r["oT"], dtype=np.float32)  # [128, H, T_pad] bf16
        sums = np.asarray(r["sums"], dtype=np.float32)[0]  # [H, T_pad]
        for h in range(HEADS_PER_CORE):
            gh = c * HEADS_PER_CORE + h
            o = (oT[:, h][:, dev_idx] / sums[h][dev_idx][None, :]).T
            out[:, gh * HEAD_DIM:(gh + 1) * HEAD_DIM] = o
    return out
